# revision 6
# baseline (speedup 1.0000x reference)
"""Cross multi-head attention TRN2 kernel (8-core SPMD, head-sharded).

Strategy (tensor parallel over heads, zero communication):
  - 16 heads / 8 cores -> 2 heads per core. Core c computes output columns
    [128*c, 128*(c+1)) of the [4096, 1024] output; host concatenates.
  - Host pre-transposes q/embed to [E, rows] and casts to bf16 so the
    contraction dim (E) lands on SBUF partitions with no on-chip transposes.
  - Scores are computed transposed (S^T[k, q] = K.Q^T, scale folded into Wq).
    The two heads per core live on partition halves 0-63 / 64-127, so their
    K=64 score matmuls map to PE row-tiles (0,0)/(64,0) and run CONCURRENTLY
    when interleaved (h0,h1,h0,h1,...) -> ~2x score throughput.
  - exp runs on ACT in 1536-wide groups (3 PSUM banks per group, double
    buffered = 6 banks) to amortize the per-ACTIVATE overhead; ACT is the
    bottleneck engine (~126us of exp per core), so the whole schedule is
    built to keep it saturated: score groups are emitted 2 ahead of their
    exp, and all other PE work (attn@V of the previous block, the other
    batch's projections, transposes) is emitted as small filler tasks
    between groups, budget-paced so the PE never starves the exp stream.
  - Softmax denominator via a ones-column appended to V (attn@V matmul also
    produces row-sums); ctx'^T is PE-transposed back to [q, d], normalized
    per-partition (DVE reciprocal+mul), and DMA'd out one block at a time.
  - Input DMAs are split into 256KB quarters round-robined over the 3
    DMA-capable queues (sync/scalar/gpsimd) so the first K/Q projections
    start ~4us in and the first exp fires ~10us in.
"""

import numpy as np
import ml_dtypes

import concourse.bass as bass
import concourse.bacc as bacc
import concourse.mybir as mybir
import concourse.tile as tile
from concourse.bass_utils import run_bass_kernel_spmd
from concourse.masks import make_identity

# ---- problem dims (hardcoded; kernel.py must be self-contained) ----
B, S, E = 2, 2048, 1024
NHEAD, HD = 16, 64
NCORES = 8
HPC = NHEAD // NCORES          # heads per core = 2
DPC = HPC * HD                 # projection out-dims per core = 128
ROWS = B * S                   # 4096
P = 128                        # SBUF partitions
NFREE = 512                    # matmul moving free dim (one PSUM bank fp32)
EC = E // P                    # 8 contraction chunks
KC = S // P                    # 16 key chunks per batch
QC = S // NFREE                # 4 query chunks per batch
RC_B = S // NFREE              # 4 projection row-chunks per batch
TPB = NFREE // P               # 4 transpose chunks per block
NSLOT = HPC * KC               # 32 score slots per (b,qc) block
GSIZES = [3] * 10 + [2]        # exp group sizes (sum = NSLOT)
NG = len(GSIZES)               # 11
SCALE = 1.0 / np.sqrt(HD)      # 0.125, folded into Wq/bq on host

F32 = mybir.dt.float32
BF16 = mybir.dt.bfloat16
AF = mybir.ActivationFunctionType

_CACHED_NC = {}
LAST_RESULTS = None            # test.py reads exec_time_ns / profile from here


class _Task:
    """A filler work item: gate = earliest group index it may be emitted
    after; deadline = group index before which it MUST be emitted (for
    program-order correctness of the in-order PE queue); cost = PE ns
    estimate for budget pacing."""

    __slots__ = ("gate", "deadline", "cost", "fn")

    def __init__(self, gate, deadline, cost, fn):
        self.gate = gate
        self.deadline = deadline
        self.cost = cost
        self.fn = fn


def _build_nc(with_bias: bool) -> bass.Bass:
    nc = bacc.Bacc(
        "TRN2",
        target_bir_lowering=False,
        debug=False,
        num_devices=NCORES,
    )

    qT = nc.declare_dram_parameter("qT", [E, ROWS], BF16, isOutput=False)
    eT = nc.declare_dram_parameter("eT", [E, ROWS], BF16, isOutput=False)
    WqT = nc.declare_dram_parameter("WqT", [E, DPC], BF16, isOutput=False)
    WkT = nc.declare_dram_parameter("WkT", [E, DPC], BF16, isOutput=False)
    WvT = nc.declare_dram_parameter("WvT", [E, DPC], BF16, isOutput=False)
    bqs = nc.declare_dram_parameter("bqs", [DPC], BF16, isOutput=False)
    bkp = nc.declare_dram_parameter("bkp", [DPC], BF16, isOutput=False)
    bvp = nc.declare_dram_parameter("bvp", [DPC], BF16, isOutput=False)
    out = nc.declare_dram_parameter("out", [ROWS, DPC], F32, isOutput=True)

    with tile.TileContext(nc) as tc:
        with (
            tc.tile_pool(name="consts", bufs=1) as consts,
            tc.tile_pool(name="wpool", bufs=1) as wpool,
            tc.tile_pool(name="resid", bufs=1) as resid,
            tc.tile_pool(name="esrc", bufs=16) as esrc,
            tc.tile_pool(name="qsrc", bufs=8) as qsrc,
            tc.tile_pool(name="prp", bufs=2) as prp,
            tc.tile_pool(name="misc", bufs=2) as misc,
            tc.tile_pool(name="otp", bufs=2) as otp,
            # PSUM: 2*3 (sp) + 1 (ctx) + 1 (proj/transpose) = 8 banks
            tc.tile_pool(name="spp", bufs=2, space="PSUM") as spp,
            tc.tile_pool(name="pctx", bufs=1, space="PSUM") as pctx,
            tc.tile_pool(name="psmall", bufs=1, space="PSUM") as psmall,
        ):
            # ---------- constants & weights (gpsimd DMA queue) ----------
            wq_sb = wpool.tile([P, EC, DPC], BF16)
            nc.gpsimd.dma_start(wq_sb, WqT.ap().rearrange("(c p) d -> p c d", p=P))
            wk_sb = wpool.tile([P, EC, DPC], BF16)
            nc.gpsimd.dma_start(wk_sb, WkT.ap().rearrange("(c p) d -> p c d", p=P))
            wv_sb = wpool.tile([P, EC, DPC], BF16)
            nc.gpsimd.dma_start(wv_sb, WvT.ap().rearrange("(c p) d -> p c d", p=P))

            ident = consts.tile([P, P], F32)
            make_identity(nc, ident)
            ones_row = consts.tile([1, NFREE], BF16)
            nc.vector.memset(ones_row, 1.0)
            # warm the ACT exp table while input DMAs stream
            warm = consts.tile([1, 1], BF16)
            nc.scalar.activation(warm, ones_row[:, 0:1], AF.Exp)

            bq_sb = wpool.tile([1, DPC], BF16)
            nc.gpsimd.dma_start(bq_sb, bqs.ap()[None, :])
            bk_sb = wpool.tile([1, DPC], BF16)
            nc.gpsimd.dma_start(bk_sb, bkp.ap()[None, :])
            bv_sb = wpool.tile([1, DPC], BF16)
            nc.gpsimd.dma_start(bv_sb, bvp.ap()[None, :])

            # ---------- residents (per batch) ----------
            qt_sb = []
            kt_sb = []
            v_sb = []
            for b in range(B):
                qt = resid.tile([P, S], BF16, name=f"qt{b}")
                kt = resid.tile([P, S], BF16, name=f"kt{b}")
                vv = resid.tile([P, KC, HPC, HD + 1], BF16, name=f"v{b}")
                nc.vector.memset(vv[:, :, :, HD : HD + 1], 1.0)
                qt_sb.append(qt)
                kt_sb.append(kt)
                v_sb.append(vv)

            # ---------- source DMAs: 256KB quarters, 3 queues ----------
            esrc_t = {}
            qsrc_t = {}
            rings = [nc.sync, nc.scalar, nc.gpsimd]
            ring_i = [0]

            def dma_src(b, r, which, use_scalar):
                dram, pool, tag, store = (
                    (qT, qsrc, "qs", qsrc_t)
                    if which == "q"
                    else (eT, esrc, "es", esrc_t)
                )
                tiles = []
                col0 = b * S + r * NFREE
                for qq in range(4):
                    tl = pool.tile(
                        [P, 2, NFREE], BF16, tag=tag, name=f"{tag}{b}_{r}_{qq}"
                    )
                    while True:
                        eng = rings[ring_i[0] % 3]
                        ring_i[0] += 1
                        if use_scalar or eng is not nc.scalar:
                            break
                    eng.dma_start(
                        tl,
                        dram.ap()[
                            qq * 2 * P : (qq + 1) * 2 * P, col0 : col0 + NFREE
                        ].rearrange("(c p) n -> p c n", p=P),
                    )
                    tiles.append(tl)
                store[(b, r)] = tiles

            def sl(tiles, c):
                return tiles[c // 2][:, c % 2]

            # ---------- projections ----------
            pp_live = {}

            def qk_proj(b, r, which, pool, ec_lo, ec_hi):
                src = (qsrc_t if which == "q" else esrc_t)[(b, r)]
                if which == "q":
                    w_t, b_t, dst = wq_sb, bq_sb, qt_sb[b]
                else:
                    w_t, b_t, dst = wk_sb, bk_sb, kt_sb[b]
                tag = "sp" if pool is spp else "ps"
                key = (b, r, which)
                if ec_lo == 0:
                    pp_live[key] = pool.tile(
                        [P, NFREE], F32, tag=tag, name=f"pp{which}{b}_{r}"
                    )
                pp = pp_live[key]
                for c in range(ec_lo, ec_hi):
                    nc.tensor.matmul(
                        pp,
                        lhsT=w_t[:, c],
                        rhs=sl(src, c),
                        start=(c == 0),
                        stop=(not with_bias and c == EC - 1),
                    )
                if ec_hi == EC:
                    if with_bias:
                        nc.tensor.matmul(
                            pp, lhsT=b_t, rhs=ones_row, start=False, stop=True
                        )
                    nc.vector.tensor_copy(dst[:, r * NFREE : (r + 1) * NFREE], pp)
                    del pp_live[key]

            def v_proj(b, r, pool, half):
                src = esrc_t[(b, r)]
                tag = "sp" if pool is spp else "ps"
                for sub in (2 * half, 2 * half + 1):
                    kc = r * TPB + sub
                    pv = pool.tile([P, DPC], F32, tag=tag, name=f"pv{b}_{kc}")
                    for c in range(EC):
                        nc.tensor.matmul(
                            pv,
                            lhsT=sl(src, c)[:, sub * P : (sub + 1) * P],
                            rhs=wv_sb[:, c],
                            start=(c == 0),
                            stop=(not with_bias and c == EC - 1),
                        )
                    if with_bias:
                        nc.tensor.matmul(
                            pv,
                            lhsT=ones_row[:, :P],
                            rhs=bv_sb,
                            start=False,
                            stop=True,
                        )
                    for h in range(HPC):
                        nc.vector.tensor_copy(
                            v_sb[b][:, kc, h, 0:HD], pv[:, h * HD : (h + 1) * HD]
                        )

            # ---------- attention ----------
            def make_ctx_tasks(b, qc, pr, slot_of, gates=None):
                """attn@V + transpose + normalize for one finished block as
                filler tasks. gates: per-task earliest group (for inlining
                into the block that produces pr); default all 0 (the tasks
                run during the NEXT block)."""
                ot = otp.tile([P, TPB, DPC], F32, tag="ot", name=f"ot{b}_{qc}")
                ctxps = {}
                ctxT = {}

                def ctx_q(h, lo):
                    def run():
                        if lo == 0:
                            ctxps[h] = pctx.tile(
                                [HD + 1, NFREE], F32, tag="ctx",
                                name=f"ctx{b}_{qc}_{h}",
                            )
                        cp = ctxps[h]
                        for kc in range(lo, lo + KC // 4):
                            nc.tensor.matmul(
                                cp,
                                lhsT=v_sb[b][:, kc, h, :],
                                rhs=pr[:, slot_of(h, kc), :],
                                start=(kc == 0),
                                stop=(kc == KC - 1),
                            )
                    return run

                def drain(h):
                    def run():
                        ctxT[h] = misc.tile(
                            [HD + 1, NFREE], F32, tag="ctxT",
                            name=f"ctxT{b}_{qc}_{h}",
                        )
                        nc.vector.tensor_copy(ctxT[h], ctxps[h])
                    return run

                def norm(h, dma):
                    def run():
                        tp = psmall.tile(
                            [P, TPB, HD + 1], F32, tag="ps",
                            name=f"tp{b}_{qc}_{h}",
                        )
                        for t in range(TPB):
                            nc.tensor.transpose(
                                tp[:, t, :],
                                ctxT[h][:, t * P : (t + 1) * P],
                                ident[: HD + 1, : HD + 1],
                            )
                        for t in range(TPB):
                            rcp = misc.tile(
                                [P, 1], F32, tag="rcp", bufs=4,
                                name=f"rcp{b}_{qc}_{h}_{t}",
                            )
                            nc.vector.reciprocal(rcp, tp[:, t, HD : HD + 1])
                            nc.vector.tensor_mul(
                                ot[:, t, h * HD : (h + 1) * HD],
                                tp[:, t, 0:HD],
                                rcp.broadcast_to([P, HD]),
                            )
                        if dma:
                            row0 = b * S + qc * NFREE
                            nc.sync.dma_start(
                                out.ap()[row0 : row0 + NFREE, :].rearrange(
                                    "(t p) d -> p t d", p=P
                                ),
                                ot,
                            )
                    return run

                fns = [
                    (ctx_q(0, 0), 900), (ctx_q(0, 4), 900),
                    (ctx_q(0, 8), 900), (ctx_q(0, 12), 900),
                    (drain(0), 100), (norm(0, False), 400),
                    (ctx_q(1, 0), 900), (ctx_q(1, 4), 900),
                    (ctx_q(1, 8), 900), (ctx_q(1, 12), 900),
                    (drain(1), 100), (norm(1, True), 400),
                ]
                if gates is None:
                    gates = [0] * len(fns)
                return [
                    _Task(g, NG - 2 if g == 0 else 99, c, f)
                    for g, (f, c) in zip(gates, fns)
                ]

            def attn_block(b, qc, pr, slot_of, slots, fillers):
                """Score+exp stream for one (batch, 512-query) block.
                Scores run 2 groups ahead of exp; fillers are emitted
                between groups under a budget so ACT never starves."""
                col0 = qc * NFREE
                groups = []
                i = 0
                for gs in GSIZES:
                    groups.append(slots[i : i + gs])
                    i += gs

                def emit_scores(g):
                    sp = spp.tile(
                        [P, 3, NFREE], F32, tag="sp", name=f"sp{b}_{qc}_{g}"
                    )
                    for j, (h, kc) in enumerate(groups[g]):
                        d0 = h * HD
                        nc.tensor.matmul(
                            sp[:, j, :],
                            lhsT=kt_sb[b][d0 : d0 + HD, kc * P : (kc + 1) * P],
                            rhs=qt_sb[b][d0 : d0 + HD, col0 : col0 + NFREE],
                            start=True,
                            stop=True,
                        )
                    return sp

                sps = {0: emit_scores(0), 1: emit_scores(1)}
                total_cost = sum(t.cost for t in fillers)
                done_cost = 0.0
                for g in range(NG):
                    grp = groups[g]
                    s0 = sum(len(x) for x in groups[:g])
                    nc.scalar.activation(
                        pr[:, s0 : s0 + len(grp), :],
                        sps.pop(g)[:, 0 : len(grp), :],
                        AF.Exp,
                    )
                    # deadline-forced pops must precede scores(g+2): those
                    # matmuls may read kt columns a K-proj task writes
                    while fillers and fillers[0].deadline <= g + 2:
                        t = fillers.pop(0)
                        t.fn()
                        done_cost += t.cost
                    if g + 2 < NG:
                        sps[g + 2] = emit_scores(g + 2)
                    want = total_cost * (g + 1) / NG
                    while (
                        fillers
                        and fillers[0].gate <= g
                        and done_cost < want
                    ):
                        t = fillers.pop(0)
                        t.fn()
                        done_cost += t.cost
                # leftovers (tail for the final block)
                while fillers:
                    fillers.pop(0).fn()

            # ---------- program ----------
            # startup: q00/e00 quarters first (3 rings), then the rest of
            # batch 0; Q0+K0 projections; attention starts ~10us in.
            dma_src(0, 0, "q", True)
            dma_src(0, 0, "e", True)
            for r in (1, 2, 3):
                dma_src(0, r, "e", True)
            for r in (1, 2, 3):
                dma_src(0, r, "q", True)
            qk_proj(0, 0, "q", spp, 0, EC)
            qk_proj(0, 0, "e", spp, 0, EC)

            def dma_b1():
                for r in range(RC_B):
                    dma_src(1, r, "e", False)
                for r in range(RC_B):
                    dma_src(1, r, "q", False)

            def T(gate, deadline, cost, fn):
                return _Task(gate, deadline, cost, fn)

            QK_COST = 850
            V_COST = 900

            # proj filler tasks per block. Deadlines: K(b) halves must emit
            # before the first score group that reads those kt columns
            # (group of slot 2*kc); Q(0,r)/K(1,*) must finish within their
            # block (default deadline NG-2 via leftover pop).
            proj_fill = {
                (0, 0): [
                    T(0, 2, QK_COST, lambda: qk_proj(0, 1, "e", psmall, 0, 4)),
                    T(0, 2, QK_COST, lambda: qk_proj(0, 1, "e", psmall, 4, EC)),
                    T(1, 5, QK_COST, lambda: qk_proj(0, 2, "e", psmall, 0, 4)),
                    T(1, 5, QK_COST, lambda: qk_proj(0, 2, "e", psmall, 4, EC)),
                    T(2, 8, QK_COST, lambda: qk_proj(0, 3, "e", psmall, 0, 4)),
                    T(2, 8, QK_COST, lambda: qk_proj(0, 3, "e", psmall, 4, EC)),
                    T(3, 9, 0, dma_b1),
                    T(3, 9, QK_COST, lambda: qk_proj(0, 1, "q", psmall, 0, 4)),
                    T(3, 9, QK_COST, lambda: qk_proj(0, 1, "q", psmall, 4, EC)),
                    T(4, 9, V_COST, lambda: v_proj(0, 0, psmall, 0)),
                    T(4, 9, V_COST, lambda: v_proj(0, 0, psmall, 1)),
                    T(5, 9, V_COST, lambda: v_proj(0, 1, psmall, 0)),
                ],
                (0, 1): [
                    T(0, 9, V_COST, lambda: v_proj(0, 1, psmall, 1)),
                    T(0, 9, V_COST, lambda: v_proj(0, 2, psmall, 0)),
                    T(0, 9, V_COST, lambda: v_proj(0, 2, psmall, 1)),
                    T(0, 9, V_COST, lambda: v_proj(0, 3, psmall, 0)),
                    T(0, 9, V_COST, lambda: v_proj(0, 3, psmall, 1)),
                    T(0, 9, QK_COST, lambda: qk_proj(0, 2, "q", psmall, 0, 4)),
                    T(0, 9, QK_COST, lambda: qk_proj(0, 2, "q", psmall, 4, EC)),
                ],
                (0, 2): [
                    T(0, 9, QK_COST, lambda: qk_proj(0, 3, "q", psmall, 0, 4)),
                    T(0, 9, QK_COST, lambda: qk_proj(0, 3, "q", psmall, 4, EC)),
                    T(0, 9, QK_COST, lambda: qk_proj(1, 0, "e", psmall, 0, 4)),
                    T(0, 9, QK_COST, lambda: qk_proj(1, 0, "e", psmall, 4, EC)),
                ],
                (0, 3): [
                    T(0, 9, QK_COST, lambda: qk_proj(1, 1, "e", psmall, 0, 4)),
                    T(0, 9, QK_COST, lambda: qk_proj(1, 1, "e", psmall, 4, EC)),
                    T(0, 9, QK_COST, lambda: qk_proj(1, 2, "e", psmall, 0, 4)),
                    T(0, 9, QK_COST, lambda: qk_proj(1, 2, "e", psmall, 4, EC)),
                    T(0, 9, QK_COST, lambda: qk_proj(1, 3, "e", psmall, 0, 4)),
                    T(0, 9, QK_COST, lambda: qk_proj(1, 3, "e", psmall, 4, EC)),
                    T(0, 9, QK_COST, lambda: qk_proj(1, 0, "q", psmall, 0, 4)),
                    T(0, 9, QK_COST, lambda: qk_proj(1, 0, "q", psmall, 4, EC)),
                ],
                (1, 0): [
                    T(0, 9, V_COST, lambda: v_proj(1, 0, psmall, 0)),
                    T(0, 9, V_COST, lambda: v_proj(1, 0, psmall, 1)),
                    T(0, 9, V_COST, lambda: v_proj(1, 1, psmall, 0)),
                    T(0, 9, V_COST, lambda: v_proj(1, 1, psmall, 1)),
                    T(0, 9, QK_COST, lambda: qk_proj(1, 1, "q", psmall, 0, 4)),
                    T(0, 9, QK_COST, lambda: qk_proj(1, 1, "q", psmall, 4, EC)),
                ],
                (1, 1): [
                    T(0, 9, V_COST, lambda: v_proj(1, 2, psmall, 0)),
                    T(0, 9, V_COST, lambda: v_proj(1, 2, psmall, 1)),
                    T(0, 9, V_COST, lambda: v_proj(1, 3, psmall, 0)),
                    T(0, 9, V_COST, lambda: v_proj(1, 3, psmall, 1)),
                    T(0, 9, QK_COST, lambda: qk_proj(1, 2, "q", psmall, 0, 4)),
                    T(0, 9, QK_COST, lambda: qk_proj(1, 2, "q", psmall, 4, EC)),
                ],
                (1, 2): [
                    T(0, 9, QK_COST, lambda: qk_proj(1, 3, "q", psmall, 0, 4)),
                    T(0, 9, QK_COST, lambda: qk_proj(1, 3, "q", psmall, 4, EC)),
                ],
                (1, 3): [],
            }

            order = [(0, 0), (0, 1), (0, 2), (0, 3), (1, 0), (1, 1), (1, 2), (1, 3)]
            prev_tasks = []
            for bi, (b, qc) in enumerate(order):
                last = bi == len(order) - 1
                pr = prp.tile([P, NSLOT, NFREE], BF16, tag="pr", name=f"pr{b}_{qc}")
                if last:
                    # head-serial slots so h0's ctx can inline into this block
                    slots = [(h, kc) for h in range(HPC) for kc in range(KC)]
                    slot_of = lambda h, kc: h * KC + kc
                else:
                    # head-paired slots -> PE row-tile concurrency
                    slots = [(h, kc) for kc in range(KC) for h in range(HPC)]
                    slot_of = lambda h, kc: kc * HPC + h
                fillers = list(proj_fill[(b, qc)]) + prev_tasks
                if last:
                    # inline gates: ctx (h, kc lo..lo+3) needs exps of slots
                    # h*16+lo+3 -> ready after group (h*16+lo+3)//3
                    gates = [2, 3, 4, 6, 6, 7, 7, 8, 9, 99, 99, 99]
                    fillers = fillers + make_ctx_tasks(b, qc, pr, slot_of, gates)
                attn_block(b, qc, pr, slot_of, slots, fillers)
                if not last:
                    prev_tasks = make_ctx_tasks(b, qc, pr, slot_of)

    nc.finalize()
    return nc


def _get_nc(with_bias: bool = True) -> bass.Bass:
    if with_bias not in _CACHED_NC:
        _CACHED_NC[with_bias] = _build_nc(with_bias)
    return _CACHED_NC[with_bias]


def kernel(embed, q, Wk, bk, Wq, bq, Wv, bv, trace=False):
    global LAST_RESULTS
    bf = ml_dtypes.bfloat16
    embed = np.asarray(embed, dtype=np.float32)
    q = np.asarray(q, dtype=np.float32)
    Wk = np.asarray(Wk, dtype=np.float32)
    Wq = np.asarray(Wq, dtype=np.float32)
    Wv = np.asarray(Wv, dtype=np.float32)
    bk = np.asarray(bk, dtype=np.float32)
    bq = np.asarray(bq, dtype=np.float32)
    bv = np.asarray(bv, dtype=np.float32)

    qT = np.ascontiguousarray(q.reshape(ROWS, E).T).astype(bf)
    eT = np.ascontiguousarray(embed.reshape(ROWS, E).T).astype(bf)

    in_maps = []
    for c in range(NCORES):
        sl = slice(c * DPC, (c + 1) * DPC)
        in_maps.append(
            {
                "qT": qT,
                "eT": eT,
                # scores scale folded into Wq/bq (exact: *2^-3)
                "WqT": np.ascontiguousarray((Wq[sl] * SCALE).T).astype(bf),
                "WkT": np.ascontiguousarray(Wk[sl].T).astype(bf),
                "WvT": np.ascontiguousarray(Wv[sl].T).astype(bf),
                "bqs": (bq[sl] * SCALE).astype(bf),
                "bkp": bk[sl].astype(bf),
                "bvp": bv[sl].astype(bf),
            }
        )

    with_bias = bool(bq.any() or bk.any() or bv.any())
    nc = _get_nc(with_bias)
    res = run_bass_kernel_spmd(nc, in_maps, list(range(NCORES)), trace=trace)
    LAST_RESULTS = res

    full = np.empty((ROWS, E), dtype=np.float32)
    for c in range(NCORES):
        full[:, c * DPC : (c + 1) * DPC] = res.results[c]["out"]
    return full.reshape(B, S, E)
